# revision 48
# baseline (speedup 1.0000x reference)
"""Trainium2 Bass kernel for nn_MultiHeadAttention_3796751090171 (sparse_attention).

Batch-parallel SPMD across 8 NeuronCores: q_batch/k_batch are SORTED, so the
cross-batch mask makes attention block-diagonal over batches, and there are
exactly B=8 batches for 8 cores. Core c computes batch c's queries against
batch c's keys for ALL 8 heads -- no collectives; the full output is a pure
row-concatenation of the per-core outputs. Uniform SPMD program with padded
shapes NQ = max_c nq_c, NK = max_c nk_c (rounded to 8).

Per-core dataflow (bf16 operands, fp32 PSUM):
  QT/KT = (Wq/s)^T-chunks @ qfT etc -> [d(128-pair), NQ/NK] (head pair t
       holds head 2t in partitions 0:64, head 2t+1 in 64:128)
  V in [k, (h, ones64|d)] layout: stationary for stage2 is
       [ones(64) | V_h(64)], so psum rows 0:64 = Z (replicated x64, already
       partition-aligned for the normalize) and rows 64:128 = hT.
  stage1 (scores^T): per head pair, kc tile: K_h^T-chunk @ Q_h with 64-row
       PE tiling -> both heads' matmuls run CONCURRENTLY in the array.
       Scores land in 2-bank psum groups [128, 2kc, NQ].
  exp: ONE activation per 2-bank group; posc = exp(pos) multiplied in on DVE
       (bf16 2x mode); masked/padded k rows have posc=0 -> contribute 0.
  stage2 -> psum [Z x64 | hT]; normalize reads PSUM directly:
       r[0:64] = 1/Z via DVE reciprocal (64 partitions), hTn = hT * r.
       No gpsimd PartitionBroadcast, no psum->sbuf staging copy.
  outproj: wo-chunk^T @ hTn accumulated in psum; oc0/oc1 chains held across
       head pairs (incremental), oc2/oc3 accumulate pairs 0..2 in stage1's
       freed banks with only the pair-3 matmul on the tail path.

DMA model: one dma_start's descriptors spread over all 16 SDMA engines; the
engines round-robin between the 3 DGE rings (sync/scalar HWDGE, gpsimd
SWDGE) at packet granularity, so each busy ring gets ~1/3 of the ~358 GB/s
HBM-per-core limit. All host arrays are pre-arranged partition-major
(contiguous multi-KB line per partition). Chunks are issued per ring in
consumption order: features/weights first (split in 2 chunks each so the
first projection matmuls start ~4us in), posc heads striped across all
three rings to keep aggregate bandwidth on the posc stream until the end.

HAM: a chain of dummy matmuls keeps the PE busy from program start so the
1.2->2.4 GHz un-throttle happens during the initial DMA wait, and the first
exp is preceded by a dummy exp so the ~1.3us ACT table load is hidden; the
PE program order then keeps activity continuous so the core never
re-throttles mid-kernel.
"""

import functools
import math

import numpy as np
import ml_dtypes

import concourse.bass as bass
import concourse.tile as tile
from concourse import bacc, mybir
from concourse.bass_utils import run_bass_kernel_spmd

N = 3072
QD = 512
OD = 512
H = 8
D = 64
B = 8
NCORES = 8
SCALE = math.sqrt(D)

F32 = mybir.dt.float32
BF16 = mybir.dt.bfloat16
BF16_NP = ml_dtypes.bfloat16

TRACE = False
LAST_RESULTS = None

KT_T = QD // 128  # contraction tiles for the projections


def _bounds(q_batch, k_batch):
    qb = np.asarray(q_batch).astype(np.int64)
    kb = np.asarray(k_batch).astype(np.int64)
    qbound = np.searchsorted(qb, np.arange(B + 1))
    kbound = np.searchsorted(kb, np.arange(B + 1))
    return qbound, kbound


def _ceil(x, m):
    return ((x + m - 1) // m) * m


@functools.lru_cache(maxsize=8)
def _build(NQ, NK, has_bq, has_bk, has_bv, has_bo):
    assert NQ <= 512 and NK <= 512
    nc = bacc.Bacc("TRN2", target_bir_lowering=False, debug=False,
                   num_devices=NCORES)

    NKC = (NK + 127) // 128          # k tiles (last may be partial)
    NKP = 128 * NKC                  # posc/vf row padding
    # kc exp groups: pairs of consecutive k tiles sharing one 2-bank psum
    KGRP = [(2 * g, min(2, NKC - 2 * g)) for g in range((NKC + 1) // 2)]

    qfT_d = nc.dram_tensor("qfT", [128, KT_T, NQ], BF16, kind="ExternalInput")
    kfT_d = nc.dram_tensor("kfT", [128, KT_T, NK], BF16, kind="ExternalInput")
    vfT_d = nc.dram_tensor("vfT", [128, KT_T, NKP], BF16, kind="ExternalInput")
    posc_d = nc.dram_tensor("posc", [H, 128, NKC * NQ], BF16,
                            kind="ExternalInput")
    # wq/wk are laid out [128, 2(td-half), KT_T, 256] so the td01 block
    # (all the early proj chains need) is ONE contiguous-per-partition DMA
    wq_d = nc.dram_tensor("wq", [128, 2, KT_T, OD // 2], BF16, kind="ExternalInput")
    wk_d = nc.dram_tensor("wk", [128, 2, KT_T, OD // 2], BF16, kind="ExternalInput")
    wv_d = nc.dram_tensor("wv", [128, KT_T, OD], BF16, kind="ExternalInput")
    wo_d = nc.dram_tensor("wo", [128, KT_T, KT_T, 128], BF16, kind="ExternalInput")
    bq_d = nc.dram_tensor("bq", [128, KT_T], F32, kind="ExternalInput") if has_bq else None
    bk_d = nc.dram_tensor("bk", [128, KT_T], F32, kind="ExternalInput") if has_bk else None
    bv_d = nc.dram_tensor("bv", [128, KT_T], F32, kind="ExternalInput") if has_bv else None
    bo_d = nc.dram_tensor("bo", [128, KT_T], F32, kind="ExternalInput") if has_bo else None
    out_d = nc.dram_tensor("out", [128, KT_T, NQ], BF16, kind="ExternalOutput")

    with tile.TileContext(nc) as tc:
        with (
            tc.tile_pool(name="consts", bufs=1) as consts,
            tc.tile_pool(name="expp", bufs=8) as expp,
            tc.tile_pool(name="pmp", bufs=10) as pmp,
            tc.tile_pool(name="zp", bufs=4) as zp,
            tc.tile_pool(name="ps_s1", bufs=2, space="PSUM") as ps_s1,
            tc.tile_pool(name="ps_s2", bufs=2, space="PSUM") as ps_s2,
            tc.tile_pool(name="ps_po", bufs=2, space="PSUM") as ps_po,
            tc.tile_pool(name="dram", bufs=1, space="DRAM") as dramp,
        ):
            # ---------- phase 0: constants + HAM warmup ----------
            junk = consts.tile([128, 128], BF16, name="junk")
            nc.vector.memset(junk, 0.5)
            tiny = consts.tile([1, 2], F32, name="tiny")
            nc.vector.memset(tiny, 0.25)
            tiny2 = consts.tile([1, 2], BF16, name="tiny2")

            zcol = consts.tile([128, 1], BF16, name="zcol")
            nc.vector.memset(zcol, 0.0)

            # HAM warmup: keep PE busy from program start until features land
            warm_ps = ps_s2.tile([128, 512], F32, tag="ps2", name="warm_ps")
            NWARM = 50
            for wi in range(NWARM):
                nc.tensor.matmul(warm_ps[:, 0:128], junk[:, :], junk[:, :],
                                 start=(wi == 0), stop=(wi == NWARM - 1))
            warm_sb = consts.tile([1, 1], F32, name="warm_sb")
            nc.vector.tensor_copy(warm_sb[0:1, 0:1], warm_ps[0:1, 0:1])
            warm_d = dramp.tile([1, 1], F32)

            # ---------- SBUF input tiles ----------
            wq_sb = consts.tile([128, 2, KT_T, OD // 2], BF16, name="wq_sb")
            wk_sb = consts.tile([128, 2, KT_T, OD // 2], BF16, name="wk_sb")
            wv_sb = consts.tile([128, KT_T, OD], BF16, name="wv_sb")
            qf_sb = consts.tile([128, KT_T, NQ], BF16, name="qf_sb")
            kf_sb = consts.tile([128, KT_T, NK], BF16, name="kf_sb")
            vf_sb = consts.tile([128, KT_T, NKP], BF16, name="vf_sb")
            wo_sb = consts.tile([128, KT_T, KT_T, 128], BF16, name="wo_sb")
            # one posc tile [128, H, NKC, NQ]: posmul reads a strided
            # (h, kc) 4D AP spanning both heads of a pair
            posc_sb = consts.tile([128, H, NKC, NQ], BF16, name="posc_sb")

            def _posc_ap(h):
                return posc_d.ap()[h].rearrange("p (kc q) -> p kc q", q=NQ)

            # ---------- DMA issue: 3 rings, consumption order ----------
            # The 16 SDMA engines round-robin across the rings-with-work,
            # so each busy ring gets ~1/3 of HBM bandwidth. The q/k proj
            # inputs gate the exp chain (the critical path), so their
            # chunks go FIRST on all three rings; posc/v/wo after.
            TH = KT_T // 2
            # sync (HWDGE ring 1): V-chain inputs FIRST (the scheduler
            # statically places the V matmuls before the K projection, so
            # their data must land first or the in-order PE head-of-line
            # blocks on them), then q/k features, then posc by deadline.
            nc.sync.dma_start(out=vf_sb[:, 0:TH], in_=vfT_d.ap()[:, 0:TH])
            nc.sync.dma_start(out=qf_sb[:, 0:TH], in_=qfT_d.ap()[:, 0:TH])
            nc.sync.dma_start(out=kf_sb[:, 0:TH], in_=kfT_d.ap()[:, 0:TH])
            bias_sb = {}
            for nm, dd in (("bq", bq_d), ("bk", bk_d), ("bv", bv_d), ("bo", bo_d)):
                if dd is not None:
                    t_ = consts.tile([128, KT_T], F32, name=f"b_{nm}")
                    nc.sync.dma_start(out=t_, in_=dd[:, :])
                    bias_sb[nm] = t_
            nc.sync.dma_start(out=posc_sb[:, 0], in_=_posc_ap(0))
            nc.sync.dma_start(out=posc_sb[:, 3], in_=_posc_ap(3))
            nc.sync.dma_start(out=posc_sb[:, 6], in_=_posc_ap(6))
            nc.sync.dma_start(out=wo_sb[:, TH:KT_T], in_=wo_d.ap()[:, TH:KT_T])

            # scalar (HWDGE ring 2): dummy exp right after the first issue
            # so the ACT table load overlaps DMA.
            nc.scalar.dma_start(out=wv_sb[:, 0:TH], in_=wv_d.ap()[:, 0:TH])
            nc.scalar.activation(tiny2[0:1, 0:2], tiny[0:1, 0:2],
                                 mybir.ActivationFunctionType.Exp)
            nc.scalar.dma_start(out=wq_sb[:, 0], in_=wq_d.ap()[:, 0])
            nc.scalar.dma_start(out=wk_sb[:, 0], in_=wk_d.ap()[:, 0])
            nc.scalar.dma_start(out=wq_sb[:, 1], in_=wq_d.ap()[:, 1])
            nc.scalar.dma_start(out=posc_sb[:, 1], in_=_posc_ap(1))
            nc.scalar.dma_start(out=posc_sb[:, 4], in_=_posc_ap(4))
            nc.scalar.dma_start(out=posc_sb[:, 7], in_=_posc_ap(7))

            # gpsimd (SWDGE ring 3)
            nc.gpsimd.dma_start(out=vf_sb[:, TH:KT_T], in_=vfT_d.ap()[:, TH:KT_T])
            nc.gpsimd.dma_start(out=wv_sb[:, TH:KT_T], in_=wv_d.ap()[:, TH:KT_T])
            nc.gpsimd.dma_start(out=qf_sb[:, TH:KT_T], in_=qfT_d.ap()[:, TH:KT_T])
            nc.gpsimd.dma_start(out=kf_sb[:, TH:KT_T], in_=kfT_d.ap()[:, TH:KT_T])
            nc.gpsimd.dma_start(out=wk_sb[:, 1], in_=wk_d.ap()[:, 1])
            nc.gpsimd.dma_start(out=wo_sb[:, 0:TH], in_=wo_d.ap()[:, 0:TH])
            nc.gpsimd.dma_start(out=posc_sb[:, 2], in_=_posc_ap(2))
            nc.gpsimd.dma_start(out=posc_sb[:, 5], in_=_posc_ap(5))

            # ---------- projected tensors ----------
            QT_f = consts.tile([128, KT_T, NQ], BF16, name="QT_f")
            KT_f = consts.tile([128, KT_T, NKP], BF16, name="KT_f")
            if NKP > NK:
                # zero the k tail so padded stage1 rows read 0 (-> exp=1,
                # then posc=0 kills them)
                nc.vector.memset(KT_f[:, :, NK:NKP], 0.0)
            # V_sb[:, kc, h, :] = [ones(64) | V_h(64)]: stage2 psum rows
            # 0:64 = Z replicated, rows 64:128 = hT. vf is host-padded to
            # NKP so V rows past nk are exact zeros (no garbage stationary).
            V_sb = consts.tile([128, NKC, H, 128], BF16, name="V_sb")
            nc.gpsimd.memset(V_sb[:, :, :, 0:D], 1.0)
            nc.gpsimd.dma_start(out=warm_d[:, :], in_=warm_sb[0:1, 0:1])
            hTn_sb = consts.tile([128, KT_T, NQ], BF16, name="hTn_sb")

            # ---------- projections (chains over t so DMA overlaps) ----------
            def proj_pair(w_sb, f_sb, bias, dst, nfree, tds, on_scalar):
                chains = [ps_po.tile([128, 512], F32, tag="po", name=f"pj{td}")
                          for td in tds]
                for t in range(KT_T):
                    for ci, td in enumerate(tds):
                        nc.tensor.matmul(
                            chains[ci][:, 0:nfree],
                            w_sb[:, td // 2, t, 128 * (td % 2):128 * (td % 2 + 1)],
                            f_sb[:, t, 0:nfree],
                            start=(t == 0), stop=(t == KT_T - 1))
                for ci, td in enumerate(tds):
                    if bias is not None:
                        nc.scalar.activation(dst[:, td, 0:nfree],
                                             chains[ci][:, 0:nfree],
                                             mybir.ActivationFunctionType.Identity,
                                             bias=bias[:, td:td + 1])
                    elif on_scalar:
                        nc.scalar.copy(dst[:, td, 0:nfree],
                                       chains[ci][:, 0:nfree])
                    else:
                        nc.vector.tensor_copy(dst[:, td, 0:nfree],
                                              chains[ci][:, 0:nfree])

            # V directly in [k, (h, ones|d)] layout: psum[kc, 512] =
            # vfT-chunk^T @ Wv; vf k-columns are padded to NKP so every
            # chain is full 128 rows.
            def v_chains(kcs):
                chains = [(kc, ps_s2.tile([128, 512], F32, tag="ps2",
                                          name=f"vch{kc}")) for kc in kcs]
                for t in range(KT_T):
                    for kc, ps in chains:
                        nc.tensor.matmul(ps[:, :],
                                         vf_sb[:, t, 128 * kc:128 * (kc + 1)],
                                         wv_sb[:, t, :],
                                         start=(t == 0), stop=(t == KT_T - 1))
                for kc, ps in chains:
                    if bias_sb.get("bv") is not None:
                        for td in range(KT_T):
                            nc.scalar.activation(
                                V_sb[:, kc, 2 * td, D:2 * D],
                                ps[:, 128 * td:128 * td + D],
                                mybir.ActivationFunctionType.Identity,
                                bias=bias_sb["bv"][:, td:td + 1])
                            nc.scalar.activation(
                                V_sb[:, kc, 2 * td + 1, D:2 * D],
                                ps[:, 128 * td + D:128 * (td + 1)],
                                mybir.ActivationFunctionType.Identity,
                                bias=bias_sb["bv"][:, td:td + 1])
                    else:
                        nc.vector.tensor_copy(
                            V_sb[:, kc, :, D:2 * D],
                            ps[:, :].rearrange("p (h d) -> p h d", d=D))

            # ---------- attention ----------
            # stage1 round = (pair, kc): ONE 2-bank psum tile, slab 0 =
            # head even, slab 1 = head odd (adjacent banks, so the 64-row
            # matmuls still run concurrently). With bufs=2 the next
            # round's matmuls overlap this round's exp, so the s1->exp
            # chain runs at the ACT exp rate. Some posmuls go to gpsimd
            # (all-SBUF op) to unload DVE.
            pm_tiles = {}   # (h, g) -> posmul output tile
            e_tiles = {}

            def stage1_pair(p):
                h, hp = 2 * p, 2 * p + 1
                t = p
                for g, gw in KGRP:
                    Sh = ps_s1.tile([128, 2, 512], F32, tag="s1", name=f"s1a{p}{g}")
                    Shp = ps_s1.tile([128, 2, 512], F32, tag="s1", name=f"s1b{p}{g}")
                    for j in range(gw):
                        kc = g + j
                        ksl = slice(128 * kc, 128 * (kc + 1))
                        nc.tensor.matmul(Sh[:, j, 0:NQ],
                                         KT_f[0:D, t, ksl], QT_f[0:D, t, 0:NQ],
                                         start=True, stop=True)
                        nc.tensor.matmul(Shp[:, j, 0:NQ],
                                         KT_f[D:128, t, ksl], QT_f[D:128, t, 0:NQ],
                                         start=True, stop=True)
                    for hh, S in ((h, Sh), (hp, Shp)):
                        E = expp.tile([128, 2, NQ], BF16, tag="e", name=f"e{hh}{g}")
                        nc.scalar.activation(E[:, 0:gw, :], S[:, 0:gw, 0:NQ],
                                             mybir.ActivationFunctionType.Exp)
                        P = pmp.tile([128, 2, NQ], BF16, tag="pm", name=f"pm{hh}{g}")
                        nc.vector.tensor_mul(P[:, 0:gw, :], E[:, 0:gw, :],
                                             posc_sb[:, hh, g:g + gw, :])
                        pm_tiles[(hh, g)] = P

            def stage2_head(h):
                S2 = ps_s2.tile([128, 512], F32, tag="ps2", name=f"s2_{h}")
                for g, gw in KGRP:
                    P = pm_tiles[(h, g)]
                    for j in range(gw):
                        kc = g + j
                        nc.tensor.matmul(S2[:, 0:NQ], V_sb[:, kc, h, :],
                                         P[:, j, :],
                                         start=(kc == 0), stop=(kc == NKC - 1))
                for g, gw in KGRP:
                    del pm_tiles[(h, g)]
                return S2

            def zpath_head(h, S2, tail=False):
                # psum rows 0:64 = Z (x64), rows 64:128 = hT. Normalize
                # straight out of PSUM: r = 1/Z on 64 partitions, then
                # hTn = hT * r (one PSUM operand per op). The multiply
                # runs on gpsimd except on the tail-critical last pair.
                po = D * (h % 2)
                r = zp.tile([D, NQ], F32, tag="zr", name=f"zr{h}")
                nc.vector.reciprocal_approx_fast(r[:, :], S2[0:D, 0:NQ])
                nc.vector.tensor_mul(hTn_sb[po:po + D, h // 2, 0:NQ],
                                     S2[D:128, 0:NQ], r[:, :])

            def emit_outproj(p, ocs, chains):
                for ci, oc in enumerate(ocs):
                    nc.tensor.matmul(chains[ci][:, 0:NQ],
                                     wo_sb[:, p, oc, :], hTn_sb[:, p, 0:NQ],
                                     start=(p == 0), stop=(p == H // 2 - 1))

            # ---------- PE program order ----------
            # preamble: Q01, K01, s1p0, V, s1p1, then the steady loop.
            proj_pair(wq_sb, qf_sb, bias_sb.get("bq"), QT_f, NQ, (0, 1),
                      on_scalar=False)
            proj_pair(wk_sb, kf_sb, bias_sb.get("bk"), KT_f, NK, (0, 1),
                      on_scalar=False)
            stage1_pair(0)
            v_chains(list(range(min(2, NKC))))
            if NKC > 2:
                v_chains(list(range(2, NKC)))
            stage1_pair(1)
            s2a = stage2_head(0)
            zpath_head(0, s2a)
            s2b = stage2_head(1)
            zpath_head(1, s2b)
            # td23 proj copies split across DVE and ACT so neither the
            # exp chain (ACT) nor the posmul stream (DVE) eats the full
            # 2.4us mid-chain
            proj_pair(wq_sb, qf_sb, bias_sb.get("bq"), QT_f, NQ, (2, 3),
                      on_scalar=False)
            proj_pair(wk_sb, kf_sb, bias_sb.get("bk"), KT_f, NK, (2, 3),
                      on_scalar=True)

            # outproj oc0/oc1 accumulate incrementally across pairs (held
            # psum; allocated only after the last proj chain released "po")
            po_oc = [ps_po.tile([128, 512], F32, tag="po", name=f"oc{i_}")
                     for i_ in range(2)]

            for p in range(1, H // 2):
                last = (p == H // 2 - 1)
                s2a = stage2_head(2 * p)
                zpath_head(2 * p, s2a, tail=last)
                s2b = stage2_head(2 * p + 1)
                zpath_head(2 * p + 1, s2b, tail=last)
                if p + 1 < H // 2:
                    stage1_pair(p + 1)
                # outproj for pair p-1: its hTn is ready; keeps PE from
                # head-of-line stalling on pair p's z-path
                emit_outproj(p - 1, (0, 1), po_oc)
            emit_outproj(H // 2 - 1, (0, 1), po_oc)

            # oc2/oc3 in stage1's freed banks: pairs 0..2 accumulate once
            # the pair-3 stage1 exps release the slots; only the pair-3
            # matmuls sit on the tail's critical path
            po_oc23 = [ps_s1.tile([128, 2, 512], F32, tag="s1",
                                  name=f"oc23_{i_}") for i_ in range(2)]
            for p in range(H // 2 - 1):
                for ci, oc in enumerate((2, 3)):
                    nc.tensor.matmul(po_oc23[ci][:, 0, 0:NQ],
                                     wo_sb[:, p, oc, :], hTn_sb[:, p, 0:NQ],
                                     start=(p == 0), stop=False)
            for ci, oc in enumerate((2, 3)):
                nc.tensor.matmul(po_oc23[ci][:, 0, 0:NQ],
                                 wo_sb[:, H // 2 - 1, oc, :],
                                 hTn_sb[:, H // 2 - 1, 0:NQ],
                                 start=False, stop=True)

            # output copies + DMA, split by oc pair so the oc01 half ships
            # while oc23 finishes
            o_sb = consts.tile([128, KT_T, NQ], BF16, name="o_sb")
            for oc in range(KT_T):
                ps = po_oc[oc][:, 0:NQ] if oc < 2 else po_oc23[oc - 2][:, 0, 0:NQ]
                if bo_d is not None:
                    nc.scalar.activation(o_sb[:, oc, :], ps,
                                         mybir.ActivationFunctionType.Identity,
                                         bias=bias_sb["bo"][:, oc:oc + 1])
                elif oc % 2 == 0:
                    nc.vector.tensor_copy(o_sb[:, oc, :], ps)
                else:
                    nc.scalar.copy(o_sb[:, oc, :], ps)
                if oc == 1:
                    nc.sync.dma_start(out=out_d.ap()[:, 0:2],
                                      in_=o_sb[:, 0:2])
            nc.scalar.dma_start(out=out_d.ap()[:, 2:KT_T],
                                in_=o_sb[:, 2:KT_T])

    nc.compile()
    return nc


def _kernel_numpy(q_feat, k_feat, v_feat, pos_enc, Wq, bq, Wk, bk, Wv, bv,
                  Wo, bo, q_batch, k_batch):
    """Host fallback (degenerate batch layouts)."""
    Q = (q_feat @ Wq + bq).reshape(N, H, D).transpose(1, 0, 2)
    K = (k_feat @ Wk + bk).reshape(N, H, D).transpose(1, 0, 2)
    V = (v_feat @ Wv + bv).reshape(N, H, D).transpose(1, 0, 2)
    scores = np.einsum("hnd,hmd->hnm", Q, K) / SCALE + pos_enc
    mask = q_batch[:, None] != k_batch[None, :]
    scores = np.where(mask[None], np.float32(-1e9), scores)
    scores = scores - scores.max(-1, keepdims=True)
    e = np.exp(scores)
    probs = e / e.sum(-1, keepdims=True)
    h = np.einsum("hnm,hmd->hnd", probs, V)
    h = h.transpose(1, 0, 2).reshape(N, OD)
    return (h @ Wo + bo).astype(np.float32)


def kernel(q_feat, k_feat, v_feat, pos_enc, Wq, bq, Wk, bk, Wv, bv, Wo, bo,
           q_batch, k_batch):
    global LAST_RESULTS
    args = dict(q_feat=np.asarray(q_feat, np.float32),
                k_feat=np.asarray(k_feat, np.float32),
                v_feat=np.asarray(v_feat, np.float32),
                pos_enc=np.asarray(pos_enc, np.float32),
                Wq=np.asarray(Wq, np.float32), bq=np.asarray(bq, np.float32),
                Wk=np.asarray(Wk, np.float32), bk=np.asarray(bk, np.float32),
                Wv=np.asarray(Wv, np.float32), bv=np.asarray(bv, np.float32),
                Wo=np.asarray(Wo, np.float32), bo=np.asarray(bo, np.float32),
                q_batch=np.asarray(q_batch), k_batch=np.asarray(k_batch))

    qbound, kbound = _bounds(args["q_batch"], args["k_batch"])
    nq_all = np.diff(qbound)
    nk_all = np.diff(kbound)
    if np.any((nq_all > 0) & (nk_all == 0)):
        return _kernel_numpy(**args)

    NQ = _ceil(max(8, int(nq_all.max())), 8)
    NK = _ceil(max(8, int(nk_all.max())), 8)
    if NQ > 512 or NK > 512:
        return _kernel_numpy(**args)
    NKP = 128 * ((NK + 127) // 128)

    has_bq = bool(np.any(args["bq"]))
    has_bk = bool(np.any(args["bk"]))
    has_bv = bool(np.any(args["bv"]))
    has_bo = bool(np.any(args["bo"]))

    nc = _build(NQ, NK, has_bq, has_bk, has_bv, has_bo)

    # ---- host-side sharding / layout / padding ----
    qfT = np.ascontiguousarray(args["q_feat"].T).astype(BF16_NP)
    kfT = np.ascontiguousarray(args["k_feat"].T).astype(BF16_NP)
    vfT = np.ascontiguousarray(args["v_feat"].T).astype(BF16_NP)

    def _pm(w):
        # [512, n] -> partition-major [128, 4, n]
        return np.ascontiguousarray(
            w.reshape(KT_T, 128, w.shape[1]).transpose(1, 0, 2))

    def _pm_qk(w):
        # [512, 512] -> [128, 2(td-half), KT_T, 256]
        return np.ascontiguousarray(
            w.reshape(KT_T, 128, 2, OD // 2).transpose(1, 2, 0, 3))

    wq8 = _pm_qk((args["Wq"] / SCALE).astype(BF16_NP))
    wkb = _pm_qk(args["Wk"].astype(BF16_NP))
    wvb = _pm(args["Wv"].astype(BF16_NP))
    wob = np.ascontiguousarray(
        args["Wo"].astype(BF16_NP).reshape(KT_T, 128, KT_T, 128)
        .transpose(1, 0, 2, 3))

    def _biascol(b, scale=1.0):
        return np.ascontiguousarray(
            (b.astype(np.float32) * scale).reshape(OD // 128, 128).T)

    in_maps = []
    for c in range(NCORES):
        qs, qe = int(qbound[c]), int(qbound[c + 1])
        ks, ke = int(kbound[c]), int(kbound[c + 1])
        nq, nk = qe - qs, ke - ks

        qfc = np.zeros((QD, NQ), BF16_NP)
        qfc[:, :nq] = qfT[:, qs:qe]
        kfc = np.zeros((QD, NK), BF16_NP)
        kfc[:, :nk] = kfT[:, ks:ke]
        vfc = np.zeros((QD, NKP), BF16_NP)
        vfc[:, :nk] = vfT[:, ks:ke]
        qfc, kfc, vfc = _pm(qfc), _pm(kfc), _pm(vfc)

        # posc holds exp(pos): 0 on masked/pad k rows, 1 on pad q cols.
        # DRAM layout is partition-major [H, 128, NKC*NQ] so each DMA
        # partition line is one contiguous descriptor.
        posc = np.zeros((H, NKP, NQ), BF16_NP)
        if nk > 0:
            posc[:, :nk, :] = 1.0
            posc[:, :nk, :nq] = np.exp(args["pos_enc"][:, qs:qe, ks:ke]) \
                .swapaxes(1, 2).astype(BF16_NP)
        posc = np.ascontiguousarray(
            posc.reshape(H, NKP // 128, 128, NQ).transpose(0, 2, 1, 3)
        ).reshape(H, 128, NKP // 128 * NQ)

        m = {"qfT": qfc, "kfT": kfc, "vfT": vfc, "posc": posc,
             "wq": wq8, "wk": wkb, "wv": wvb, "wo": wob}
        if has_bq:
            m["bq"] = _biascol(args["bq"], 1.0 / SCALE)
        if has_bk:
            m["bk"] = _biascol(args["bk"])
        if has_bv:
            m["bv"] = _biascol(args["bv"])
        if has_bo:
            m["bo"] = _biascol(args["bo"])
        in_maps.append(m)

    res = run_bass_kernel_spmd(nc, in_maps, core_ids=list(range(NCORES)),
                               trace=TRACE)
    LAST_RESULTS = res
    out = np.empty((N, OD), np.float32)
    for c in range(NCORES):
        qs, qe = int(qbound[c]), int(qbound[c + 1])
        if qe > qs:
            arr = res.results[c]["out"]  # [128, KT_T, NQ] partition-major
            full = arr.transpose(1, 0, 2).reshape(OD, -1)
            out[qs:qe, :] = full[:, :qe - qs].T.astype(np.float32)
    return out


# revision 49
# speedup vs baseline: 1.0073x; 1.0073x over previous
"""Trainium2 Bass kernel for nn_MultiHeadAttention_3796751090171 (sparse_attention).

Batch-parallel SPMD across 8 NeuronCores: q_batch/k_batch are SORTED, so the
cross-batch mask makes attention block-diagonal over batches, and there are
exactly B=8 batches for 8 cores. Core c computes batch c's queries against
batch c's keys for ALL 8 heads -- no collectives; the full output is a pure
row-concatenation of the per-core outputs. Uniform SPMD program with padded
shapes NQ = max_c nq_c, NK = max_c nk_c (rounded to 8).

Per-core dataflow (bf16 operands, fp32 PSUM):
  QT/KT = (Wq/s)^T-chunks @ qfT etc -> [d(128-pair), NQ/NK] (head pair t
       holds head 2t in partitions 0:64, head 2t+1 in 64:128)
  V in [k, (h, ones64|d)] layout: stationary for stage2 is
       [ones(64) | V_h(64)], so psum rows 0:64 = Z (replicated x64, already
       partition-aligned for the normalize) and rows 64:128 = hT.
  stage1 (scores^T): per head pair, kc tile: K_h^T-chunk @ Q_h with 64-row
       PE tiling -> both heads' matmuls run CONCURRENTLY in the array.
       Scores land in 2-bank psum groups [128, 2kc, NQ].
  exp: ONE activation per 2-bank group; posc = exp(pos) multiplied in on DVE
       (bf16 2x mode); masked/padded k rows have posc=0 -> contribute 0.
  stage2 -> psum [Z x64 | hT]; normalize reads PSUM directly:
       r[0:64] = 1/Z via DVE reciprocal (64 partitions), hTn = hT * r.
       No gpsimd PartitionBroadcast, no psum->sbuf staging copy.
  outproj: wo-chunk^T @ hTn accumulated in psum; oc0/oc1 chains held across
       head pairs (incremental), oc2/oc3 accumulate pairs 0..2 in stage1's
       freed banks with only the pair-3 matmul on the tail path.

DMA model: one dma_start's descriptors spread over all 16 SDMA engines; the
engines round-robin between the 3 DGE rings (sync/scalar HWDGE, gpsimd
SWDGE) at packet granularity, so each busy ring gets ~1/3 of the ~358 GB/s
HBM-per-core limit. All host arrays are pre-arranged partition-major
(contiguous multi-KB line per partition). Chunks are issued per ring in
consumption order: features/weights first (split in 2 chunks each so the
first projection matmuls start ~4us in), posc heads striped across all
three rings to keep aggregate bandwidth on the posc stream until the end.

HAM: a chain of dummy matmuls keeps the PE busy from program start so the
1.2->2.4 GHz un-throttle happens during the initial DMA wait, and the first
exp is preceded by a dummy exp so the ~1.3us ACT table load is hidden; the
PE program order then keeps activity continuous so the core never
re-throttles mid-kernel.
"""

import functools
import math

import numpy as np
import ml_dtypes

import concourse.bass as bass
import concourse.tile as tile
from concourse import bacc, mybir
from concourse.bass_utils import run_bass_kernel_spmd

N = 3072
QD = 512
OD = 512
H = 8
D = 64
B = 8
NCORES = 8
SCALE = math.sqrt(D)

F32 = mybir.dt.float32
BF16 = mybir.dt.bfloat16
BF16_NP = ml_dtypes.bfloat16

TRACE = False
LAST_RESULTS = None

KT_T = QD // 128  # contraction tiles for the projections


def _bounds(q_batch, k_batch):
    qb = np.asarray(q_batch).astype(np.int64)
    kb = np.asarray(k_batch).astype(np.int64)
    qbound = np.searchsorted(qb, np.arange(B + 1))
    kbound = np.searchsorted(kb, np.arange(B + 1))
    return qbound, kbound


def _ceil(x, m):
    return ((x + m - 1) // m) * m


@functools.lru_cache(maxsize=8)
def _build(NQ, NK, has_bq, has_bk, has_bv, has_bo):
    assert NQ <= 512 and NK <= 512
    nc = bacc.Bacc("TRN2", target_bir_lowering=False, debug=False,
                   num_devices=NCORES)

    NKC = (NK + 127) // 128          # k tiles (last may be partial)
    NKP = 128 * NKC                  # posc/vf row padding
    # kc exp groups: pairs of consecutive k tiles sharing one 2-bank psum
    KGRP = [(2 * g, min(2, NKC - 2 * g)) for g in range((NKC + 1) // 2)]

    qfT_d = nc.dram_tensor("qfT", [128, KT_T, NQ], BF16, kind="ExternalInput")
    kfT_d = nc.dram_tensor("kfT", [128, KT_T, NK], BF16, kind="ExternalInput")
    vfT_d = nc.dram_tensor("vfT", [128, KT_T, NKP], BF16, kind="ExternalInput")
    posc_d = nc.dram_tensor("posc", [H, 128, NKC * NQ], BF16,
                            kind="ExternalInput")
    # wq/wk are laid out [128, 2(td-half), KT_T, 256] so the td01 block
    # (all the early proj chains need) is ONE contiguous-per-partition DMA
    wq_d = nc.dram_tensor("wq", [128, 2, KT_T, OD // 2], BF16, kind="ExternalInput")
    wk_d = nc.dram_tensor("wk", [128, 2, KT_T, OD // 2], BF16, kind="ExternalInput")
    wv_d = nc.dram_tensor("wv", [128, KT_T, OD], BF16, kind="ExternalInput")
    wo_d = nc.dram_tensor("wo", [128, KT_T, KT_T, 128], BF16, kind="ExternalInput")
    bq_d = nc.dram_tensor("bq", [128, KT_T], F32, kind="ExternalInput") if has_bq else None
    bk_d = nc.dram_tensor("bk", [128, KT_T], F32, kind="ExternalInput") if has_bk else None
    bv_d = nc.dram_tensor("bv", [128, KT_T], F32, kind="ExternalInput") if has_bv else None
    bo_d = nc.dram_tensor("bo", [128, KT_T], F32, kind="ExternalInput") if has_bo else None
    out_d = nc.dram_tensor("out", [128, KT_T, NQ], BF16, kind="ExternalOutput")

    with tile.TileContext(nc) as tc:
        with (
            tc.tile_pool(name="consts", bufs=1) as consts,
            tc.tile_pool(name="expp", bufs=8) as expp,
            tc.tile_pool(name="pmp", bufs=10) as pmp,
            tc.tile_pool(name="zp", bufs=4) as zp,
            tc.tile_pool(name="ps_s1", bufs=2, space="PSUM") as ps_s1,
            tc.tile_pool(name="ps_s2", bufs=2, space="PSUM") as ps_s2,
            tc.tile_pool(name="ps_po", bufs=2, space="PSUM") as ps_po,
            tc.tile_pool(name="dram", bufs=1, space="DRAM") as dramp,
        ):
            # ---------- phase 0: constants + HAM warmup ----------
            junk = consts.tile([128, 128], BF16, name="junk")
            nc.vector.memset(junk, 0.5)
            tiny = consts.tile([1, 2], F32, name="tiny")
            nc.vector.memset(tiny, 0.25)
            tiny2 = consts.tile([1, 2], BF16, name="tiny2")

            zcol = consts.tile([128, 1], BF16, name="zcol")
            nc.vector.memset(zcol, 0.0)

            # HAM warmup: keep PE busy from program start until features land
            warm_ps = ps_s2.tile([128, 512], F32, tag="ps2", name="warm_ps")
            NWARM = 30
            for wi in range(NWARM):
                nc.tensor.matmul(warm_ps[:, 0:128], junk[:, :], junk[:, :],
                                 start=(wi == 0), stop=(wi == NWARM - 1))
            warm_sb = consts.tile([1, 1], F32, name="warm_sb")
            nc.vector.tensor_copy(warm_sb[0:1, 0:1], warm_ps[0:1, 0:1])
            warm_d = dramp.tile([1, 1], F32)

            # ---------- SBUF input tiles ----------
            wq_sb = consts.tile([128, 2, KT_T, OD // 2], BF16, name="wq_sb")
            wk_sb = consts.tile([128, 2, KT_T, OD // 2], BF16, name="wk_sb")
            wv_sb = consts.tile([128, KT_T, OD], BF16, name="wv_sb")
            qf_sb = consts.tile([128, KT_T, NQ], BF16, name="qf_sb")
            kf_sb = consts.tile([128, KT_T, NK], BF16, name="kf_sb")
            vf_sb = consts.tile([128, KT_T, NKP], BF16, name="vf_sb")
            wo_sb = consts.tile([128, KT_T, KT_T, 128], BF16, name="wo_sb")
            # one posc tile [128, H, NKC, NQ]: posmul reads a strided
            # (h, kc) 4D AP spanning both heads of a pair
            posc_sb = consts.tile([128, H, NKC, NQ], BF16, name="posc_sb")

            def _posc_ap(h):
                return posc_d.ap()[h].rearrange("p (kc q) -> p kc q", q=NQ)

            # ---------- DMA issue: 3 rings, consumption order ----------
            # The 16 SDMA engines round-robin across the rings-with-work,
            # so each busy ring gets ~1/3 of HBM bandwidth. The q/k proj
            # inputs gate the exp chain (the critical path), so their
            # chunks go FIRST on all three rings; posc/v/wo after.
            TH = KT_T // 2
            # sync (HWDGE ring 1): V-chain inputs FIRST (the scheduler
            # statically places the V matmuls before the K projection, so
            # their data must land first or the in-order PE head-of-line
            # blocks on them), then q/k features, then posc by deadline.
            nc.sync.dma_start(out=vf_sb[:, 0:TH], in_=vfT_d.ap()[:, 0:TH])
            nc.sync.dma_start(out=qf_sb[:, 0:TH], in_=qfT_d.ap()[:, 0:TH])
            nc.sync.dma_start(out=kf_sb[:, 0:TH], in_=kfT_d.ap()[:, 0:TH])
            bias_sb = {}
            for nm, dd in (("bq", bq_d), ("bk", bk_d), ("bv", bv_d), ("bo", bo_d)):
                if dd is not None:
                    t_ = consts.tile([128, KT_T], F32, name=f"b_{nm}")
                    nc.sync.dma_start(out=t_, in_=dd[:, :])
                    bias_sb[nm] = t_
            nc.sync.dma_start(out=posc_sb[:, 0], in_=_posc_ap(0))
            nc.sync.dma_start(out=posc_sb[:, 3], in_=_posc_ap(3))
            nc.sync.dma_start(out=posc_sb[:, 6], in_=_posc_ap(6))
            nc.sync.dma_start(out=wo_sb[:, TH:KT_T], in_=wo_d.ap()[:, TH:KT_T])

            # scalar (HWDGE ring 2): dummy exp right after the first issue
            # so the ACT table load overlaps DMA.
            nc.scalar.dma_start(out=wv_sb[:, 0:TH], in_=wv_d.ap()[:, 0:TH])
            nc.scalar.activation(tiny2[0:1, 0:2], tiny[0:1, 0:2],
                                 mybir.ActivationFunctionType.Exp)
            nc.scalar.dma_start(out=wq_sb[:, 0], in_=wq_d.ap()[:, 0])
            nc.scalar.dma_start(out=wk_sb[:, 0], in_=wk_d.ap()[:, 0])
            nc.scalar.dma_start(out=wq_sb[:, 1], in_=wq_d.ap()[:, 1])
            nc.scalar.dma_start(out=posc_sb[:, 1], in_=_posc_ap(1))
            nc.scalar.dma_start(out=posc_sb[:, 4], in_=_posc_ap(4))
            nc.scalar.dma_start(out=posc_sb[:, 7], in_=_posc_ap(7))

            # gpsimd (SWDGE ring 3)
            nc.gpsimd.dma_start(out=vf_sb[:, TH:KT_T], in_=vfT_d.ap()[:, TH:KT_T])
            nc.gpsimd.dma_start(out=wv_sb[:, TH:KT_T], in_=wv_d.ap()[:, TH:KT_T])
            nc.gpsimd.dma_start(out=qf_sb[:, TH:KT_T], in_=qfT_d.ap()[:, TH:KT_T])
            nc.gpsimd.dma_start(out=kf_sb[:, TH:KT_T], in_=kfT_d.ap()[:, TH:KT_T])
            nc.gpsimd.dma_start(out=wk_sb[:, 1], in_=wk_d.ap()[:, 1])
            nc.gpsimd.dma_start(out=wo_sb[:, 0:TH], in_=wo_d.ap()[:, 0:TH])
            nc.gpsimd.dma_start(out=posc_sb[:, 2], in_=_posc_ap(2))
            nc.gpsimd.dma_start(out=posc_sb[:, 5], in_=_posc_ap(5))

            # ---------- projected tensors ----------
            QT_f = consts.tile([128, KT_T, NQ], BF16, name="QT_f")
            KT_f = consts.tile([128, KT_T, NKP], BF16, name="KT_f")
            if NKP > NK:
                # zero the k tail so padded stage1 rows read 0 (-> exp=1,
                # then posc=0 kills them)
                nc.vector.memset(KT_f[:, :, NK:NKP], 0.0)
            # V_sb[:, kc, h, :] = [ones(64) | V_h(64)]: stage2 psum rows
            # 0:64 = Z replicated, rows 64:128 = hT. vf is host-padded to
            # NKP so V rows past nk are exact zeros (no garbage stationary).
            V_sb = consts.tile([128, NKC, H, 128], BF16, name="V_sb")
            nc.gpsimd.memset(V_sb[:, :, :, 0:D], 1.0)
            nc.gpsimd.dma_start(out=warm_d[:, :], in_=warm_sb[0:1, 0:1])
            hTn_sb = consts.tile([128, KT_T, NQ], BF16, name="hTn_sb")

            # ---------- projections (chains over t so DMA overlaps) ----------
            def proj_pair(w_sb, f_sb, bias, dst, nfree, tds, on_scalar):
                chains = [ps_po.tile([128, 512], F32, tag="po", name=f"pj{td}")
                          for td in tds]
                for t in range(KT_T):
                    for ci, td in enumerate(tds):
                        nc.tensor.matmul(
                            chains[ci][:, 0:nfree],
                            w_sb[:, td // 2, t, 128 * (td % 2):128 * (td % 2 + 1)],
                            f_sb[:, t, 0:nfree],
                            start=(t == 0), stop=(t == KT_T - 1))
                for ci, td in enumerate(tds):
                    if bias is not None:
                        nc.scalar.activation(dst[:, td, 0:nfree],
                                             chains[ci][:, 0:nfree],
                                             mybir.ActivationFunctionType.Identity,
                                             bias=bias[:, td:td + 1])
                    elif on_scalar:
                        nc.scalar.copy(dst[:, td, 0:nfree],
                                       chains[ci][:, 0:nfree])
                    else:
                        nc.vector.tensor_copy(dst[:, td, 0:nfree],
                                              chains[ci][:, 0:nfree])

            # V directly in [k, (h, ones|d)] layout: psum[kc, 512] =
            # vfT-chunk^T @ Wv; vf k-columns are padded to NKP so every
            # chain is full 128 rows.
            def v_chains(kcs):
                chains = [(kc, ps_s2.tile([128, 512], F32, tag="ps2",
                                          name=f"vch{kc}")) for kc in kcs]
                for t in range(KT_T):
                    for kc, ps in chains:
                        nc.tensor.matmul(ps[:, :],
                                         vf_sb[:, t, 128 * kc:128 * (kc + 1)],
                                         wv_sb[:, t, :],
                                         start=(t == 0), stop=(t == KT_T - 1))
                for kc, ps in chains:
                    if bias_sb.get("bv") is not None:
                        for td in range(KT_T):
                            nc.scalar.activation(
                                V_sb[:, kc, 2 * td, D:2 * D],
                                ps[:, 128 * td:128 * td + D],
                                mybir.ActivationFunctionType.Identity,
                                bias=bias_sb["bv"][:, td:td + 1])
                            nc.scalar.activation(
                                V_sb[:, kc, 2 * td + 1, D:2 * D],
                                ps[:, 128 * td + D:128 * (td + 1)],
                                mybir.ActivationFunctionType.Identity,
                                bias=bias_sb["bv"][:, td:td + 1])
                    else:
                        nc.vector.tensor_copy(
                            V_sb[:, kc, :, D:2 * D],
                            ps[:, :].rearrange("p (h d) -> p h d", d=D))

            # ---------- attention ----------
            # stage1 round = (pair, kc): ONE 2-bank psum tile, slab 0 =
            # head even, slab 1 = head odd (adjacent banks, so the 64-row
            # matmuls still run concurrently). With bufs=2 the next
            # round's matmuls overlap this round's exp, so the s1->exp
            # chain runs at the ACT exp rate. Some posmuls go to gpsimd
            # (all-SBUF op) to unload DVE.
            pm_tiles = {}   # (h, g) -> posmul output tile
            e_tiles = {}

            def stage1_pair(p):
                h, hp = 2 * p, 2 * p + 1
                t = p
                for g, gw in KGRP:
                    Sh = ps_s1.tile([128, 2, 512], F32, tag="s1", name=f"s1a{p}{g}")
                    Shp = ps_s1.tile([128, 2, 512], F32, tag="s1", name=f"s1b{p}{g}")
                    for j in range(gw):
                        kc = g + j
                        ksl = slice(128 * kc, 128 * (kc + 1))
                        nc.tensor.matmul(Sh[:, j, 0:NQ],
                                         KT_f[0:D, t, ksl], QT_f[0:D, t, 0:NQ],
                                         start=True, stop=True)
                        nc.tensor.matmul(Shp[:, j, 0:NQ],
                                         KT_f[D:128, t, ksl], QT_f[D:128, t, 0:NQ],
                                         start=True, stop=True)
                    for hh, S in ((h, Sh), (hp, Shp)):
                        E = expp.tile([128, 2, NQ], BF16, tag="e", name=f"e{hh}{g}")
                        nc.scalar.activation(E[:, 0:gw, :], S[:, 0:gw, 0:NQ],
                                             mybir.ActivationFunctionType.Exp)
                        P = pmp.tile([128, 2, NQ], BF16, tag="pm", name=f"pm{hh}{g}")
                        nc.vector.tensor_mul(P[:, 0:gw, :], E[:, 0:gw, :],
                                             posc_sb[:, hh, g:g + gw, :])
                        pm_tiles[(hh, g)] = P

            def stage2_head(h):
                S2 = ps_s2.tile([128, 512], F32, tag="ps2", name=f"s2_{h}")
                for g, gw in KGRP:
                    P = pm_tiles[(h, g)]
                    for j in range(gw):
                        kc = g + j
                        nc.tensor.matmul(S2[:, 0:NQ], V_sb[:, kc, h, :],
                                         P[:, j, :],
                                         start=(kc == 0), stop=(kc == NKC - 1))
                for g, gw in KGRP:
                    del pm_tiles[(h, g)]
                return S2

            def zpath_head(h, S2, tail=False):
                # psum rows 0:64 = Z (x64), rows 64:128 = hT. Normalize
                # straight out of PSUM: r = 1/Z on 64 partitions, then
                # hTn = hT * r (one PSUM operand per op). The multiply
                # runs on gpsimd except on the tail-critical last pair.
                po = D * (h % 2)
                r = zp.tile([D, NQ], F32, tag="zr", name=f"zr{h}")
                nc.vector.reciprocal_approx_fast(r[:, :], S2[0:D, 0:NQ])
                nc.vector.tensor_mul(hTn_sb[po:po + D, h // 2, 0:NQ],
                                     S2[D:128, 0:NQ], r[:, :])

            def emit_outproj(p, ocs, chains):
                for ci, oc in enumerate(ocs):
                    nc.tensor.matmul(chains[ci][:, 0:NQ],
                                     wo_sb[:, p, oc, :], hTn_sb[:, p, 0:NQ],
                                     start=(p == 0), stop=(p == H // 2 - 1))

            # ---------- PE program order ----------
            # preamble: Q01, K01, s1p0, V, s1p1, then the steady loop.
            proj_pair(wq_sb, qf_sb, bias_sb.get("bq"), QT_f, NQ, (0, 1),
                      on_scalar=False)
            proj_pair(wk_sb, kf_sb, bias_sb.get("bk"), KT_f, NK, (0, 1),
                      on_scalar=False)
            stage1_pair(0)
            v_chains(list(range(min(2, NKC))))
            if NKC > 2:
                v_chains(list(range(2, NKC)))
            stage1_pair(1)
            s2a = stage2_head(0)
            zpath_head(0, s2a)
            s2b = stage2_head(1)
            zpath_head(1, s2b)
            # td23 proj copies split across DVE and ACT so neither the
            # exp chain (ACT) nor the posmul stream (DVE) eats the full
            # 2.4us mid-chain
            proj_pair(wq_sb, qf_sb, bias_sb.get("bq"), QT_f, NQ, (2, 3),
                      on_scalar=False)
            proj_pair(wk_sb, kf_sb, bias_sb.get("bk"), KT_f, NK, (2, 3),
                      on_scalar=True)

            # outproj oc0/oc1 accumulate incrementally across pairs (held
            # psum; allocated only after the last proj chain released "po")
            po_oc = [ps_po.tile([128, 512], F32, tag="po", name=f"oc{i_}")
                     for i_ in range(2)]

            for p in range(1, H // 2):
                last = (p == H // 2 - 1)
                s2a = stage2_head(2 * p)
                zpath_head(2 * p, s2a, tail=last)
                s2b = stage2_head(2 * p + 1)
                zpath_head(2 * p + 1, s2b, tail=last)
                if p + 1 < H // 2:
                    stage1_pair(p + 1)
                # outproj for pair p-1: its hTn is ready; keeps PE from
                # head-of-line stalling on pair p's z-path
                emit_outproj(p - 1, (0, 1), po_oc)
            emit_outproj(H // 2 - 1, (0, 1), po_oc)

            # oc2/oc3 in stage1's freed banks: pairs 0..2 accumulate once
            # the pair-3 stage1 exps release the slots; only the pair-3
            # matmuls sit on the tail's critical path
            po_oc23 = [ps_s1.tile([128, 2, 512], F32, tag="s1",
                                  name=f"oc23_{i_}") for i_ in range(2)]
            for p in range(H // 2 - 1):
                for ci, oc in enumerate((2, 3)):
                    nc.tensor.matmul(po_oc23[ci][:, 0, 0:NQ],
                                     wo_sb[:, p, oc, :], hTn_sb[:, p, 0:NQ],
                                     start=(p == 0), stop=False)
            for ci, oc in enumerate((2, 3)):
                nc.tensor.matmul(po_oc23[ci][:, 0, 0:NQ],
                                 wo_sb[:, H // 2 - 1, oc, :],
                                 hTn_sb[:, H // 2 - 1, 0:NQ],
                                 start=False, stop=True)

            # output copies + DMA, split by oc pair so the oc01 half ships
            # while oc23 finishes
            o_sb = consts.tile([128, KT_T, NQ], BF16, name="o_sb")
            for oc in range(KT_T):
                ps = po_oc[oc][:, 0:NQ] if oc < 2 else po_oc23[oc - 2][:, 0, 0:NQ]
                if bo_d is not None:
                    nc.scalar.activation(o_sb[:, oc, :], ps,
                                         mybir.ActivationFunctionType.Identity,
                                         bias=bias_sb["bo"][:, oc:oc + 1])
                elif oc % 2 == 0:
                    nc.vector.tensor_copy(o_sb[:, oc, :], ps)
                else:
                    nc.scalar.copy(o_sb[:, oc, :], ps)
                if oc == 1:
                    nc.sync.dma_start(out=out_d.ap()[:, 0:2],
                                      in_=o_sb[:, 0:2])
            nc.scalar.dma_start(out=out_d.ap()[:, 2:KT_T],
                                in_=o_sb[:, 2:KT_T])

    nc.compile()
    return nc


def _kernel_numpy(q_feat, k_feat, v_feat, pos_enc, Wq, bq, Wk, bk, Wv, bv,
                  Wo, bo, q_batch, k_batch):
    """Host fallback (degenerate batch layouts)."""
    Q = (q_feat @ Wq + bq).reshape(N, H, D).transpose(1, 0, 2)
    K = (k_feat @ Wk + bk).reshape(N, H, D).transpose(1, 0, 2)
    V = (v_feat @ Wv + bv).reshape(N, H, D).transpose(1, 0, 2)
    scores = np.einsum("hnd,hmd->hnm", Q, K) / SCALE + pos_enc
    mask = q_batch[:, None] != k_batch[None, :]
    scores = np.where(mask[None], np.float32(-1e9), scores)
    scores = scores - scores.max(-1, keepdims=True)
    e = np.exp(scores)
    probs = e / e.sum(-1, keepdims=True)
    h = np.einsum("hnm,hmd->hnd", probs, V)
    h = h.transpose(1, 0, 2).reshape(N, OD)
    return (h @ Wo + bo).astype(np.float32)


def kernel(q_feat, k_feat, v_feat, pos_enc, Wq, bq, Wk, bk, Wv, bv, Wo, bo,
           q_batch, k_batch):
    global LAST_RESULTS
    args = dict(q_feat=np.asarray(q_feat, np.float32),
                k_feat=np.asarray(k_feat, np.float32),
                v_feat=np.asarray(v_feat, np.float32),
                pos_enc=np.asarray(pos_enc, np.float32),
                Wq=np.asarray(Wq, np.float32), bq=np.asarray(bq, np.float32),
                Wk=np.asarray(Wk, np.float32), bk=np.asarray(bk, np.float32),
                Wv=np.asarray(Wv, np.float32), bv=np.asarray(bv, np.float32),
                Wo=np.asarray(Wo, np.float32), bo=np.asarray(bo, np.float32),
                q_batch=np.asarray(q_batch), k_batch=np.asarray(k_batch))

    qbound, kbound = _bounds(args["q_batch"], args["k_batch"])
    nq_all = np.diff(qbound)
    nk_all = np.diff(kbound)
    if np.any((nq_all > 0) & (nk_all == 0)):
        return _kernel_numpy(**args)

    NQ = _ceil(max(8, int(nq_all.max())), 8)
    NK = _ceil(max(8, int(nk_all.max())), 8)
    if NQ > 512 or NK > 512:
        return _kernel_numpy(**args)
    NKP = 128 * ((NK + 127) // 128)

    has_bq = bool(np.any(args["bq"]))
    has_bk = bool(np.any(args["bk"]))
    has_bv = bool(np.any(args["bv"]))
    has_bo = bool(np.any(args["bo"]))

    nc = _build(NQ, NK, has_bq, has_bk, has_bv, has_bo)

    # ---- host-side sharding / layout / padding ----
    qfT = np.ascontiguousarray(args["q_feat"].T).astype(BF16_NP)
    kfT = np.ascontiguousarray(args["k_feat"].T).astype(BF16_NP)
    vfT = np.ascontiguousarray(args["v_feat"].T).astype(BF16_NP)

    def _pm(w):
        # [512, n] -> partition-major [128, 4, n]
        return np.ascontiguousarray(
            w.reshape(KT_T, 128, w.shape[1]).transpose(1, 0, 2))

    def _pm_qk(w):
        # [512, 512] -> [128, 2(td-half), KT_T, 256]
        return np.ascontiguousarray(
            w.reshape(KT_T, 128, 2, OD // 2).transpose(1, 2, 0, 3))

    wq8 = _pm_qk((args["Wq"] / SCALE).astype(BF16_NP))
    wkb = _pm_qk(args["Wk"].astype(BF16_NP))
    wvb = _pm(args["Wv"].astype(BF16_NP))
    wob = np.ascontiguousarray(
        args["Wo"].astype(BF16_NP).reshape(KT_T, 128, KT_T, 128)
        .transpose(1, 0, 2, 3))

    def _biascol(b, scale=1.0):
        return np.ascontiguousarray(
            (b.astype(np.float32) * scale).reshape(OD // 128, 128).T)

    in_maps = []
    for c in range(NCORES):
        qs, qe = int(qbound[c]), int(qbound[c + 1])
        ks, ke = int(kbound[c]), int(kbound[c + 1])
        nq, nk = qe - qs, ke - ks

        qfc = np.zeros((QD, NQ), BF16_NP)
        qfc[:, :nq] = qfT[:, qs:qe]
        kfc = np.zeros((QD, NK), BF16_NP)
        kfc[:, :nk] = kfT[:, ks:ke]
        vfc = np.zeros((QD, NKP), BF16_NP)
        vfc[:, :nk] = vfT[:, ks:ke]
        qfc, kfc, vfc = _pm(qfc), _pm(kfc), _pm(vfc)

        # posc holds exp(pos): 0 on masked/pad k rows, 1 on pad q cols.
        # DRAM layout is partition-major [H, 128, NKC*NQ] so each DMA
        # partition line is one contiguous descriptor.
        posc = np.zeros((H, NKP, NQ), BF16_NP)
        if nk > 0:
            posc[:, :nk, :] = 1.0
            posc[:, :nk, :nq] = np.exp(args["pos_enc"][:, qs:qe, ks:ke]) \
                .swapaxes(1, 2).astype(BF16_NP)
        posc = np.ascontiguousarray(
            posc.reshape(H, NKP // 128, 128, NQ).transpose(0, 2, 1, 3)
        ).reshape(H, 128, NKP // 128 * NQ)

        m = {"qfT": qfc, "kfT": kfc, "vfT": vfc, "posc": posc,
             "wq": wq8, "wk": wkb, "wv": wvb, "wo": wob}
        if has_bq:
            m["bq"] = _biascol(args["bq"], 1.0 / SCALE)
        if has_bk:
            m["bk"] = _biascol(args["bk"])
        if has_bv:
            m["bv"] = _biascol(args["bv"])
        if has_bo:
            m["bo"] = _biascol(args["bo"])
        in_maps.append(m)

    res = run_bass_kernel_spmd(nc, in_maps, core_ids=list(range(NCORES)),
                               trace=TRACE)
    LAST_RESULTS = res
    out = np.empty((N, OD), np.float32)
    for c in range(NCORES):
        qs, qe = int(qbound[c]), int(qbound[c + 1])
        if qe > qs:
            arr = res.results[c]["out"]  # [128, KT_T, NQ] partition-major
            full = arr.transpose(1, 0, 2).reshape(OD, -1)
            out[qs:qe, :] = full[:, :qe - qs].T.astype(np.float32)
    return out
